# revision 29
# baseline (speedup 1.0000x reference)
"""Trainium2 Bass kernel for the sparse_attention nn_Kernel problem.

Math (per sample, derived from the reference):
  t1 = p1w * x ; C_k[i,m] = sum_p x[i,p] * t1pad[m, p + (k-3)]  (c x c, 7 shifts)
  t7m_k[q=m, i] = A[q] - C_k^T[q]   where A[q] = C_3^T[q+1] (channel roll)
  Sm = sum_k t7m_k ; Sj[q] = Sm[q-1]
  out = roll_{h+1,w-1}( sum_k t7m_k^T @ shift_k(t1pad) + Sj^T @ xroll )

Layout/schedule (v4 -- gap-packed flat + shift-composition transposes):
  - t1/x live in a GAP-PACKED flat layout: rows at stride 59, the 3-zero gap
    between rows serves as BOTH the right pad of row h and the left pad of
    row h+1 (unfold pad=3 semantics for every shift).  Flat contraction
    length 3328 (26 chunks of 128) vs the baseline's 3584 (28 chunks):
    -7% bmm1 matmul columns and transpose tiles.
  - SHIFT COMPOSITION: t1T_a x xT_b contracts x[j]*t1[j+a-b] (the leading
    row-gap zeros absorb the index offset), so the 7 shifts come from just
    6 transposed tensors: t1-side {0,3} x x-side {0,1,2,3}.  The baseline
    moved 8 full transposed tensors + 12 shifted copies through the XBAR;
    this needs 2 persistent + 4 transient tensors, of which t1T3 is built
    on the TENSOR engine (identity-matmul transpose into a rotating PSUM
    bank) during the natural wait for the first x-transient.
  - All XBAR transposes stay on ONE queue (sync): concurrent XBAR
    transposes on both hwdge queues corrupt each other (shared XBAR unit).
    p1w loads / rolls / half the stores go on the scalar queue.  x/p use
    per-half tiles so load DMAs don't falsely serialize behind mul reads
    (coarse dep tracking stalls the in-order queues otherwise).
  - matmul start=True zeroes the WHOLE PSUM bank, so the two mb halves of a
    C_k pair packed into one bank carry start/stop only on the first/last
    matmul of the pair group.
  - The A-term is folded into t7 (t7_k = A - C_k), so bmm2 is 16 contraction
    chunks per out tile (14 t7 + 2 Sj) in a single PSUM session per tile,
    drained once with the output roll fused, then stored directly.  Five
    ib=0 tiles accumulate inside the k loop (1-k delay) to fill
    transient-transpose waits; the rest run as a second pass.
  - Sm = sum_k t7_k comes free (gpsimd adds); Sj/A are small partition-roll
    DMAs on the scalar queue hidden under the k loop.

Each of the 8 cores processes one sample of the batch (data parallel).
"""

import math

import numpy as np

C = 256
H = 56
W = 56
K = 7
RS = 59  # flat row stride (56 + shared 3-zero gap)
NREAL = H * RS  # 3304: muls write rows into [0, NREAL)
NCH = 26  # contraction chunks of 128
NP = NCH * 128  # 3328 padded contraction length
F0 = 3  # left slack (zeros) before flat position 0
FW = 3344  # buffer width (>= F0 + NP + 3 shift slack, zero-padded tail)
BETA = 1.0 / (math.sqrt(H * W) * math.sqrt(C * K))
N_CORES = 8
HT = 8  # h rows per bmm2 out tile
KORD = (3, 1, 0, 2, 5, 4, 6)
# shift composition: C at shift s uses lhsT = t1T_a, rhs = xT_b with s = a-b
# (t1T_a x xT_b contracts x[j] * t1[j + a - b]; the leading row-gap zeros
# absorb the j-range offset).  Shift set {0,2,3} x {0,2,3} covers all 7
# shifts with 6 tensors, only TWO of them x-side transients (XBAR); the two
# t1-side transients are built on the tensor engine.
KPAIR = {3: (0, 0), 1: (0, 2), 0: (0, 3), 2: (2, 3), 5: (2, 0), 4: (3, 2), 6: (3, 0)}
TPIECES = ((0, 9), (9, 18), (18, 26))  # persistent-transpose pieces

_CACHE = {}


def _build_nc():
    import concourse.mybir as mybir
    import concourse.tile as tile
    from concourse import bacc
    from concourse import masks

    f32 = mybir.dt.float32
    bf16 = mybir.dt.bfloat16

    nc = bacc.Bacc("TRN2", target_bir_lowering=False, debug=False)

    xin = nc.dram_tensor("x", [C, H, W], f32, kind="ExternalInput").ap()
    pwin = nc.dram_tensor("p1w", [C, H, W], f32, kind="ExternalInput").ap()
    out = nc.dram_tensor("out", [C, H, W], f32, kind="ExternalOutput").ap()

    sub = mybir.AluOpType.subtract
    add = mybir.AluOpType.add

    with tile.TileContext(nc) as tc:
        with (
            tc.tile_pool(name="fx", bufs=1) as pfx,
            tc.tile_pool(name="bf59", bufs=1) as p59,
            tc.tile_pool(name="hT", bufs=1) as phT,
            tc.tile_pool(name="hTx", bufs=2) as ptx,
            tc.tile_pool(name="roll", bufs=1) as pbr,
            tc.tile_pool(name="small", bufs=1) as psm,
            tc.tile_pool(name="osb", bufs=1) as pob,
            tc.tile_pool(name="ps1", bufs=2, space="PSUM") as pps1,
            tc.tile_pool(name="ps2", bufs=5, space="PSUM") as pps2,
            tc.tile_pool(name="pstr", bufs=1, space="PSUM") as pstr,
        ):
            # ---------------- tiles ----------------
            QR = 14  # rows per mul quarter
            x_cp, p_cp, t1b, x59, xroll = [], [], [], [], []
            for cb in range(2):
                # per-half tiles: a shared tile makes the h1-load DMA falsely
                # depend on mul reads of h0 (coarse dep tracking), stalling
                # the in-order queue and everything behind it.
                x_cp.append(
                    [
                        pfx.tile(
                            [128, H // 2, W], f32, tag=f"xcp{cb}{hb}",
                            name=f"xcp{cb}{hb}",
                        )
                        for hb in range(2)
                    ]
                )
                p_cp.append(
                    [
                        pfx.tile(
                            [128, H // 2, W], f32, tag=f"pcp{cb}{hb}",
                            name=f"pcp{cb}{hb}",
                        )
                        for hb in range(2)
                    ]
                )
                t1b.append(
                    p59.tile([128, FW], bf16, tag=f"t1b{cb}", name=f"t1b{cb}")
                )
                x59.append(
                    p59.tile([128, FW], bf16, tag=f"x59{cb}", name=f"x59{cb}")
                )
                xroll.append(
                    pbr.tile([128, H, W], bf16, tag=f"xr{cb}", name=f"xr{cb}")
                )
            t1T0 = phT.tile([128, NCH, C], bf16, tag="t1T0")
            xT0 = phT.tile([128, NCH, C], bf16, tag="xT0")
            out_sb = [
                pob.tile([128, H, W], f32, tag=f"osb{ib}", name=f"osb{ib}")
                for ib in range(2)
            ]

            # ------------- pad memsets (before loads/muls) -------------
            for cb in range(2):
                for buf, eng in ((t1b[cb], nc.vector), (x59[cb], nc.gpsimd)):
                    v3 = buf[:, F0 : F0 + NREAL].rearrange(
                        "p (h u) -> p h u", u=RS
                    )
                    eng.memset(v3[:, :, 0:3], 0.0)  # row gaps
                    eng.memset(buf[:, 0:F0], 0.0)  # left slack
                    eng.memset(buf[:, F0 + NREAL : FW], 0.0)  # tail slack

            # ------------- loads (plain contiguous fp32) -------------
            # sync: x h0 + p h1 ; scalar: p h0 + x h1
            HH = H // 2
            for cb in range(2):
                nc.sync.dma_start(
                    x_cp[cb][0][:], xin[cb * 128 : (cb + 1) * 128][:, 0:HH, :]
                )
                nc.scalar.dma_start(
                    p_cp[cb][0][:], pwin[cb * 128 : (cb + 1) * 128][:, 0:HH, :]
                )
            for cb in range(2):
                nc.scalar.dma_start(
                    x_cp[cb][1][:], xin[cb * 128 : (cb + 1) * 128][:, HH:H, :]
                )
                nc.sync.dma_start(
                    p_cp[cb][1][:], pwin[cb * 128 : (cb + 1) * 128][:, HH:H, :]
                )

            # ------------- 59-space bf16 operands -------------
            # t1 = x*p (vector), x59 = BETA*x (scalar); quarter-row pieces
            t1v = [
                t1b[cb][:, F0 : F0 + NREAL].rearrange("p (h u) -> p h u", u=RS)
                for cb in range(2)
            ]
            x5v = [
                x59[cb][:, F0 : F0 + NREAL].rearrange("p (h u) -> p h u", u=RS)
                for cb in range(2)
            ]
            for qh in range(4):
                rows = slice(qh * QR, (qh + 1) * QR)
                hb, rh = qh // 2, slice((qh % 2) * QR, (qh % 2) * QR + QR)
                for cb in range(2):
                    nc.scalar.activation(
                        x5v[cb][:, rows, 3:RS],
                        x_cp[cb][hb][:, rh, :],
                        mybir.ActivationFunctionType.Copy,
                        scale=BETA,
                    )
                    nc.vector.tensor_mul(
                        t1v[cb][:, rows, 3:RS],
                        x_cp[cb][hb][:, rh, :],
                        p_cp[cb][hb][:, rh, :],
                    )

            # ------------- persistent transposes (sync queue, XBAR) --------
            T = {("t1", 0): t1T0, ("x", 0): xT0}
            for t0, t1 in TPIECES:
                for side in ("t1", "x"):
                    srcs = x59 if side == "x" else t1b
                    for cb in range(2):
                        nc.sync.dma_start_transpose(
                            T[(side, 0)][:, t0:t1, cb * 128 : (cb + 1) * 128],
                            srcs[cb][:, F0 + 128 * t0 : F0 + 128 * t1],
                        )

            def make_shiftT(side, d, tX):
                # transposed operand shifted by d: value at flat pos 128t+q+d
                srcs = x59 if side == "x" else t1b
                for cb in range(2):
                    nc.sync.dma_start_transpose(
                        tX[:, :, cb * 128 : (cb + 1) * 128],
                        srcs[cb][:, F0 + d : F0 + d + NP],
                    )
                T[(side, d)] = tX

            # identity + psum staging for the PE transpose of t1T3 (built on
            # the tensor engine during the natural k3->k2 transient-wait gap)
            ident = psm.tile([128, 128], bf16, tag="ident")
            masks.make_identity(nc, ident[:])
            ptr_bank = pstr.tile([128, 1024], bf16, tag="ptr", name="ptr")

            def pe_transpose(dst, side, d):
                srcs = x59 if side == "x" else t1b
                for cb in range(2):
                    for t in range(NCH):
                        s = ((cb * NCH + t) % 8) * 128
                        blk = ptr_bank[:, s : s + 128]
                        nc.tensor.transpose(
                            blk,
                            srcs[cb][
                                :, F0 + d + 128 * t : F0 + d + 128 * (t + 1)
                            ],
                            ident[:],
                        )
                        nc.vector.tensor_copy(
                            out=dst[:, t, cb * 128 : (cb + 1) * 128], in_=blk
                        )
                T[(side, d)] = dst

            # ------------- bmm1 k loop (fused bmm2 partials) -------------
            C3_sb, A_bf, Sm, Sj = [], [], [], []
            for mb in range(2):
                C3_sb.append(
                    psm.tile([128, C], bf16, tag=f"c3sb{mb}", name=f"c3sb{mb}")
                )
                A_bf.append(
                    psm.tile([128, C], bf16, tag=f"abf{mb}", name=f"abf{mb}")
                )
                Sm.append(psm.tile([128, C], bf16, tag=f"sm{mb}", name=f"sm{mb}"))
                Sj.append(psm.tile([128, C], bf16, tag=f"sj{mb}", name=f"sj{mb}"))
            t7 = {}

            def rhs1(mb, k, h0):
                base = F0 + (k - 3) + 3 + RS * h0
                return t1b[mb][:, base : base + RS * HT].rearrange(
                    "p (h u) -> p h u", u=RS
                )[:, :, 0:W]

            # fused bmm2 tiles (ib=0): accumulate per-k partials inside the
            # k loop (1-k delay) to fill transpose-supply stalls
            FUSED = [(0, h0) for h0 in range(0, 40, HT)]
            qtiles = {}
            for ib, h0 in FUSED:
                q = pps2.tile([128, 2 * C], f32, tag="ps2", name=f"qf{h0}")
                qtiles[(ib, h0)] = q[:, 0 : HT * W].rearrange(
                    "p (h w) -> p h w", w=W
                )

            def bmm2_partial(k, first):
                for mb in range(2):
                    for ib, h0 in FUSED:
                        nc.tensor.matmul(
                            qtiles[(ib, h0)][:],
                            t7[(k, mb)][:, ib * 128 : (ib + 1) * 128],
                            rhs1(mb, k, h0),
                            start=(first and mb == 0),
                            stop=False,
                        )

            t1T2 = phT.tile([128, NCH, C], bf16, tag="t1T2")
            t1T3 = phT.tile([128, NCH, C], bf16, tag="t1T3")
            xt_tiles = {
                d: ptx.tile([128, NCH, C], bf16, tag="tX", name=f"xT{d}")
                for d in (2, 3)
            }
            # x-transients via XBAR up-front; t1T2/t1T3 via PE in-loop
            make_shiftT("x", 2, xt_tiles[2])
            make_shiftT("x", 3, xt_tiles[3])

            for ki, k in enumerate(KORD):
                a, b = KPAIR[k]
                lhsT, rhs = T[("t1", a)], T[("x", b)]

                # NOTE: matmul start=True zeroes the WHOLE PSUM bank, so with
                # two mb accumulation regions packed into one bank only the
                # very first matmul may carry start (and only the last stop).
                pair = pps1.tile([128, 2 * C], f32, tag="ps1", name=f"pair{k}")
                for t in range(NCH):
                    for mb in range(2):
                        nc.tensor.matmul(
                            pair[:, mb * C : (mb + 1) * C],
                            lhsT[:, t, mb * 128 : (mb + 1) * 128],
                            rhs[:, t, :],
                            start=(t == 0 and mb == 0),
                            stop=(t == NCH - 1 and mb == 1),
                            skip_group_check=True,
                        )

                if k == 3:
                    for mb in range(2):
                        nc.vector.tensor_copy(
                            out=C3_sb[mb][:],
                            in_=pair[:, mb * C : (mb + 1) * C],
                        )
                    for mb in range(2):
                        nc.scalar.dma_start(
                            A_bf[mb][0:127, :], C3_sb[mb][1:128, :]
                        )
                        nc.scalar.dma_start(
                            A_bf[mb][127:128, :], C3_sb[1 - mb][0:1, :]
                        )
                if ki == 0:
                    pe_transpose(t1T2, "t1", 2)
                if ki == 2:
                    pe_transpose(t1T3, "t1", 3)

                for mb in range(2):
                    cn = psm.tile(
                        [128, C], bf16, tag=f"t7_{k}_{mb}", name=f"t7_{k}_{mb}"
                    )
                    nc.vector.tensor_tensor(
                        cn[:], A_bf[mb][:], pair[:, mb * C : (mb + 1) * C], sub
                    )
                    t7[(k, mb)] = cn
                    if k == 3:
                        nc.gpsimd.tensor_copy(out=Sm[mb][:], in_=cn[:])
                    else:
                        nc.gpsimd.tensor_tensor(Sm[mb][:], Sm[mb][:], cn[:], add)

                # fused bmm2 partials for the previous k (its t7 is drained)
                if ki >= 1:
                    bmm2_partial(KORD[ki - 1], first=(ki == 1))

                if ki == 3:
                    # xroll[j,h',w'] = x[j,(h'+1)%H,(w'-1)%W] (Sj-term rhs).
                    # Emitted mid-loop: needed only by bmm2, and early
                    # emission contends with the XBAR/load SBUF traffic.
                    for cb in range(2):
                        xr = xroll[cb]
                        x0, x1 = x_cp[cb][0], x_cp[cb][1]
                        for dst_r, srct, src_r in (
                            (slice(0, 27), x0, slice(1, 28)),
                            (slice(27, 55), x1, slice(0, 28)),
                            (slice(55, 56), x0, slice(0, 1)),
                        ):
                            nc.vector.tensor_copy(
                                out=xr[:, dst_r, 1:W],
                                in_=srct[:, src_r, 0 : W - 1],
                            )
                            nc.vector.tensor_copy(
                                out=xr[:, dst_r, 0:1],
                                in_=srct[:, src_r, W - 1 : W],
                            )

            bmm2_partial(KORD[-1], first=False)

            for mb in range(2):
                nc.scalar.dma_start(Sj[mb][1:128, :], Sm[mb][0:127, :])
                nc.scalar.dma_start(Sj[mb][0:1, :], Sm[1 - mb][127:128, :])

            # ------------- bmm2 finish: Sj terms, drains, pass 2 -----------
            np_q = 0

            def drain_store(ib, h0, qv):
                nonlocal np_q
                use_v = np_q % 2 == 0

                def cp(o, i):
                    if use_v:
                        nc.vector.tensor_copy(out=o, in_=i)
                    else:
                        nc.scalar.copy(o, i)

                def roll_copy(r0, r1, d0):
                    cp(
                        out_sb[ib][:, d0 : d0 + (r1 - r0), 0 : W - 1],
                        qv[:, r0:r1, 1:W],
                    )
                    cp(
                        out_sb[ib][:, d0 : d0 + (r1 - r0), W - 1 : W],
                        qv[:, r0:r1, 0:1],
                    )

                if h0 + HT < H:
                    roll_copy(0, HT, h0 + 1)
                else:
                    roll_copy(0, HT - 1, h0 + 1)
                    roll_copy(HT - 1, HT, 0)

                ob = out[ib * 128 : (ib + 1) * 128]
                qeng = nc.sync if np_q % 2 == 0 else nc.scalar
                if h0 + HT < H:
                    qeng.dma_start(
                        ob[:, h0 + 1 : h0 + 1 + HT, :],
                        out_sb[ib][:, h0 + 1 : h0 + 1 + HT, :],
                    )
                else:
                    qeng.dma_start(
                        ob[:, h0 + 1 : H, :], out_sb[ib][:, h0 + 1 : H, :]
                    )
                    qeng.dma_start(ob[:, 0:1, :], out_sb[ib][:, 0:1, :])
                np_q += 1

            for ib, h0 in FUSED:
                qv = qtiles[(ib, h0)]
                for mb in range(2):
                    nc.tensor.matmul(
                        qv[:],
                        Sj[mb][:, ib * 128 : (ib + 1) * 128],
                        xroll[mb][:, h0 : h0 + HT, :],
                        start=False,
                        stop=(mb == 1),
                    )
                drain_store(ib, h0, qv)

            PASS2 = [(0, 40), (0, 48)] + [(1, h0) for h0 in range(0, H, HT)]
            for ib, h0 in PASS2:
                q = pps2.tile([128, 2 * C], f32, tag="ps2", name=f"q{ib}_{h0}")
                qv = q[:, 0 : HT * W].rearrange("p (h w) -> p h w", w=W)
                first = True
                for k in KORD:
                    for mb in range(2):
                        nc.tensor.matmul(
                            qv[:],
                            t7[(k, mb)][:, ib * 128 : (ib + 1) * 128],
                            rhs1(mb, k, h0),
                            start=first,
                            stop=False,
                        )
                        first = False
                for mb in range(2):
                    nc.tensor.matmul(
                        qv[:],
                        Sj[mb][:, ib * 128 : (ib + 1) * 128],
                        xroll[mb][:, h0 : h0 + HT, :],
                        start=False,
                        stop=(mb == 1),
                    )
                drain_store(ib, h0, qv)

    nc.compile()
    return nc


def _get_nc():
    if "nc" not in _CACHE:
        _CACHE["nc"] = _build_nc()
    return _CACHE["nc"]


def kernel(x: np.ndarray, p1w: np.ndarray) -> np.ndarray:
    from concourse.bass_utils import run_bass_kernel_spmd

    n = x.shape[0]
    assert n == N_CORES
    x = np.ascontiguousarray(np.asarray(x, dtype=np.float32))
    pw = np.ascontiguousarray(np.asarray(p1w, dtype=np.float32)[0])

    nc = _get_nc()
    in_maps = [{"x": x[i], "p1w": pw} for i in range(n)]
    res = run_bass_kernel_spmd(nc, in_maps, list(range(N_CORES)))
    outs = [res.results[i]["out"] for i in range(n)]
    return np.stack(outs, axis=0).astype(np.float32)
